# revision 2
# baseline (speedup 1.0000x reference)
"""GroupSparseAE (FISTA group-lasso encoder + linear decoder) on 8 trn2 cores.

Data-parallel over batch: each core gets B/8 = 64 rows, W replicated.
Per channel c (3 total, processed sequentially so W[c]/W[c]^T fit in SBUF):
  y2   = TAU * (W @ x^T)                   [D, b] transposed layout
  FISTA iterate k = 1..30 with x in transposed [D, b] layout:
    u^T    = W^T-contract:  uT[n,b]   = sum_d W[d,n] xT[d,b]
    grad^T = gT[e,b]        = sum_n WT[n,e] uT[n,b]
    v      = xT_tmp + y2 - TAU*gT
    group soft-threshold (groups of 8 along d = partition dim):
       gs = Bmat^T @ v^2  (Bmat block-diag ones -> broadcast group sumsq)
       xnew = relu(v) * relu(1 - c/sqrt(gs))
    momentum: xtmp = xnew + m_k (xnew - xold)
  decode: out^T[n,b] = sum_d W[d,n] z[d,b]
All matmuls: stationary [128,128] weight tile, moving [128,64] activation
slice, fp32 accumulate in PSUM.
"""

import sys

sys.path.insert(0, "/opt/trn_rl_repo")

import numpy as np

B, C, N = 512, 3, 1024
G, S = 256, 8
D = G * S  # 2048
NUM_LAYERS = 30
TAU, LAM = 0.1, 0.1
CTH = LAM * TAU  # group threshold constant

N_CORES = 8
BL = B // N_CORES  # 64 rows per core
NT = D // 128  # 16 d-tiles
NS = N // 128  # 8 n-tiles
FD = NT * BL  # 1024 flat free dim of [D, b] state
CHUNK = 256  # elementwise chunk (4 d-tiles)
NCH = FD // CHUNK


def _mom_coeffs(num_layers):
    # fp32 t-sequence to match the reference's on-device arithmetic
    one, four, two = np.float32(1.0), np.float32(4.0), np.float32(2.0)
    t = np.float32(1.0)
    ms = []
    for _ in range(num_layers):
        t_new = (one + np.sqrt(one + four * t * t)) / two
        ms.append(float((t - one) / t_new))
        t = t_new
    return ms


def _bmat_np():
    p = np.arange(128)
    return (p[:, None] // S == p[None, :] // S).astype(np.float32)


def build(num_layers=NUM_LAYERS):
    import concourse.bacc as bacc
    from concourse import mybir
    from concourse.tile import TileContext

    fp32 = mybir.dt.float32
    AF = mybir.ActivationFunctionType
    OP = mybir.AluOpType

    nc = bacc.Bacc("TRN2", target_bir_lowering=False, debug=False,
                   num_devices=N_CORES)
    xt = nc.dram_tensor("xt", [C, N, BL], fp32, kind="ExternalInput")
    w = nc.dram_tensor("w", [C, D, N], fp32, kind="ExternalInput")
    wt = nc.dram_tensor("wt", [C, N, D], fp32, kind="ExternalInput")
    bm = nc.dram_tensor("bm", [128, 128], fp32, kind="ExternalInput")
    ot = nc.dram_tensor("ot", [C, N, BL], fp32, kind="ExternalOutput")

    ms = _mom_coeffs(num_layers)

    with TileContext(nc) as tc:
        with (
            tc.tile_pool(name="wp", bufs=1) as wp,
            tc.tile_pool(name="st", bufs=1) as st,
            tc.tile_pool(name="scr", bufs=4) as scr,
            tc.tile_pool(name="ps_u", bufs=2, space="PSUM") as ps_u,
            tc.tile_pool(name="ps_g", bufs=3, space="PSUM") as ps_g,
            tc.tile_pool(name="ps_s", bufs=2, space="PSUM") as ps_s,
        ):
            bmat = wp.tile([128, 128], fp32, tag="bmat")
            nc.sync.dma_start(out=bmat, in_=bm[:, :])

            for c in range(C):
                wsb = wp.tile([128, NT, N], fp32, tag="wsb")
                nc.sync.dma_start(
                    out=wsb, in_=w[c].rearrange("(t p) n -> p t n", p=128))
                wtsb = wp.tile([128, NS, D], fp32, tag="wtsb")
                nc.sync.dma_start(
                    out=wtsb, in_=wt[c].rearrange("(s p) e -> p s e", p=128))
                xts = wp.tile([128, NS, BL], fp32, tag="xts")
                nc.sync.dma_start(
                    out=xts, in_=xt[c].rearrange("(s p) b -> p s b", p=128))

                # persistent per-channel state
                y2 = st.tile([128, FD], fp32, tag="y2")
                xb0 = st.tile([128, FD], fp32, tag="xb0")
                xb1 = st.tile([128, FD], fp32, tag="xb1")
                xbuf = [xb0, xb1]
                uT = st.tile([128, NS * BL], fp32, tag="uT")
                # chunked tiles for cross-iteration pipelining
                xtmp = [st.tile([128, CHUNK], fp32, tag=f"xtmp{j}",
                                name=f"xtmp{j}") for j in range(NCH)]
                pre = [st.tile([128, CHUNK], fp32, tag=f"pre{j}",
                               name=f"pre{j}") for j in range(NCH)]

                nc.vector.memset(xb0, 0.0)

                # ---- precomp: y2 = TAU * W @ x^T  in [D, b] layout ----
                for t in range(NT):
                    py = ps_g.tile([128, BL], fp32, tag="pg")
                    for s in range(NS):
                        nc.tensor.matmul(
                            py, wtsb[:, s, t * 128:(t + 1) * 128],
                            xts[:, s, :], start=(s == 0), stop=(s == NS - 1))
                    nc.scalar.mul(y2[:, t * BL:(t + 1) * BL], py, TAU)

                def act_block(vch, k):
                    """vch(j) -> [128, CHUNK] AP of the pre-activation v.
                    Writes xnew (xbuf[k % 2]); unless last iter, also xtmp/pre.
                    """
                    xnew, xold = xbuf[k % 2], xbuf[(k - 1) % 2]
                    m = ms[k - 1]
                    last = k == num_layers
                    for j in range(NCH):
                        sl = slice(j * CHUNK, (j + 1) * CHUNK)
                        vj = vch(j)
                        v2 = scr.tile([128, CHUNK], fp32, tag="v2")
                        nc.scalar.square(v2, vj)
                        gs = ps_s.tile([128, CHUNK], fp32, tag="gs")
                        nc.tensor.matmul(gs, bmat, v2, start=True, stop=True)
                        nrm = scr.tile([128, CHUNK], fp32, tag="nrm")
                        nc.scalar.sqrt(nrm, gs)
                        invn = scr.tile([128, CHUNK], fp32, tag="invn")
                        nc.vector.reciprocal(invn, nrm)
                        scl = scr.tile([128, CHUNK], fp32, tag="scl")
                        # relu(1 - CTH / nrm)
                        nc.scalar.activation(scl, invn, AF.Relu,
                                             bias=1.0, scale=-CTH)
                        # xnew = max(v, 0) * scl
                        nc.vector.scalar_tensor_tensor(
                            xnew[:, sl], vj, 0.0, scl,
                            op0=OP.max, op1=OP.mult)
                        if not last:
                            dd = scr.tile([128, CHUNK], fp32, tag="dd")
                            nc.vector.tensor_sub(dd, xnew[:, sl], xold[:, sl])
                            nc.vector.scalar_tensor_tensor(
                                xtmp[j], dd, m, xnew[:, sl],
                                op0=OP.mult, op1=OP.add)
                            nc.vector.tensor_add(pre[j], xtmp[j], y2[:, sl])

                # ---- iteration 1: x_tmp = 0 -> v = y2 ----
                act_block(lambda j: y2[:, j * CHUNK:(j + 1) * CHUNK], 1)

                # ---- iterations 2..num_layers ----
                for k in range(2, num_layers + 1):
                    # u-phase: uT[n,b] = sum_d W[d,n] xtmp[d,b]
                    for s in range(NS):
                        pu = ps_u.tile([128, BL], fp32, tag="pu")
                        for t in range(NT):
                            nc.tensor.matmul(
                                pu, wsb[:, t, s * 128:(s + 1) * 128],
                                xtmp[t // 4][:, (t % 4) * BL:(t % 4 + 1) * BL],
                                start=(t == 0), stop=(t == NT - 1))
                        nc.scalar.copy(uT[:, s * BL:(s + 1) * BL], pu)
                    # grad-phase + v-combine
                    vt = [scr.tile([128, CHUNK], fp32, tag=f"v{j}", name=f"v{j}")
                          for j in range(NCH)]
                    for t in range(NT):
                        pg = ps_g.tile([128, BL], fp32, tag="pg")
                        for s in range(NS):
                            nc.tensor.matmul(
                                pg, wtsb[:, s, t * 128:(t + 1) * 128],
                                uT[:, s * BL:(s + 1) * BL],
                                start=(s == 0), stop=(s == NS - 1))
                        # v = pre - TAU * grad
                        nc.vector.scalar_tensor_tensor(
                            vt[t // 4][:, (t % 4) * BL:(t % 4 + 1) * BL],
                            pg, -TAU, pre[t // 4][:, (t % 4) * BL:(t % 4 + 1) * BL],
                            op0=OP.mult, op1=OP.add)
                    act_block(lambda j: vt[j][:, :], k)

                # ---- decode: out^T[n,b] = sum_d W[d,n] z[d,b] ----
                z = xbuf[num_layers % 2]
                otsb = st.tile([128, NS, BL], fp32, tag="otsb")
                for s in range(NS):
                    pd = ps_u.tile([128, BL], fp32, tag="pu")
                    for t in range(NT):
                        nc.tensor.matmul(
                            pd, wsb[:, t, s * 128:(s + 1) * 128],
                            z[:, t * BL:(t + 1) * BL],
                            start=(t == 0), stop=(t == NT - 1))
                    nc.scalar.copy(otsb[:, s, :], pd)
                nc.sync.dma_start(
                    out=ot[c].rearrange("(s p) b -> p s b", p=128), in_=otsb)

    nc.compile()
    return nc


_CACHED = {}


def _get_nc(num_layers=NUM_LAYERS):
    if num_layers not in _CACHED:
        _CACHED[num_layers] = build(num_layers)
    return _CACHED[num_layers]


def make_in_maps(x, w):
    """x [B,C,N] fp32, w [C,D,N] fp32 -> list of 8 per-core input dicts."""
    x = np.asarray(x, dtype=np.float32)
    w = np.ascontiguousarray(np.asarray(w, dtype=np.float32))
    wt = np.ascontiguousarray(w.transpose(0, 2, 1))
    bm = _bmat_np()
    maps = []
    for i in range(N_CORES):
        xs = x[i * BL:(i + 1) * BL]  # [BL, C, N]
        xts = np.ascontiguousarray(xs.transpose(1, 2, 0))  # [C, N, BL]
        maps.append({"xt": xts, "w": w, "wt": wt, "bm": bm})
    return maps


def assemble_out(results):
    outs = []
    for i in range(N_CORES):
        o = results[i]["ot"]  # [C, N, BL]
        outs.append(np.ascontiguousarray(o.transpose(2, 0, 1)))  # [BL, C, N]
    return np.concatenate(outs, axis=0).astype(np.float32)


def kernel(x, W):
    from concourse.bass_utils import run_bass_kernel_spmd

    nc = _get_nc()
    res = run_bass_kernel_spmd(nc, make_in_maps(x, W), list(range(N_CORES)))
    return assemble_out(res.results)


if __name__ == "__main__":
    xs = np.random.randn(B, C, N).astype(np.float32)
    ws = np.random.randn(C, D, N).astype(np.float32)
    ws /= np.linalg.norm(ws, axis=-1, keepdims=True)
    out = kernel(xs, ws)
    print("out", out.shape, out.dtype, float(np.abs(out).mean()))
